# revision 1
# baseline (speedup 1.0000x reference)
"""Trainium2 Bass kernel for nn_Colar_static (retrieval_knn).

Sharding: data-parallel over batch B=2048 across 8 NeuronCores (256 rows each).
Static exemplar banks and weights are precomputed/reshaped on host and
replicated to all cores.

Per-core pipeline (all layouts keep batch in the matmul FREE dim or on
partitions as needed; j = flattened (class, exemplar) = 21*32 = 672):
  1. kvT[o,b]   = WkvT-blocks^T @ xT          (PE, bf16, K=2048)
  2. kT,vT      = psum evict (+bias, relu for v)  (ACT)
  3. sumsq[b]   = ones-matmul over kT^2       (DVE square + PE)
     rinv[b]    = 1/sqrt(sumsq)               (ACT sqrt + DVE recip)
  4. dot[b,j]   = kT-blocks^T @ Ekn_mat       (PE, K=1024)
  5. e = exp(rinv*dot)  (ACT, per-partition scale; cos in [-1,1] so no
     max-subtraction is needed for softmax stability)
  6. S,num      = blockwise reduces over 32-exemplar groups (DVE)
     t = num/S; g = exp(t); fw = g/G; c = fw/S  (class softmax; the scalar
     bias bw cancels in softmax)
  7. u[b,j] = e * c (block-broadcast)         (DVE)
  8. uT = PE-transpose(u)                     (PE + copies)
  9. fE_T[o,b]  = A_mat-blocks^T @ uT         (PE, K=672), relu evict
 10. outT[cls,b]= WoutT-blocks^T @ [relu(vT); relu(fE_T)]  (PE, K=2048)
 11. out = psum + bout -> DMA                 (DVE)

Host gathers the 8 [21,256] results into [2048, 21, 1].
"""

import numpy as np
import ml_dtypes

import concourse.bass as bass
import concourse.bacc as bacc
import concourse.mybir as mybir
import concourse.tile as tile
from concourse.bass_utils import run_bass_kernel_spmd

AF = mybir.ActivationFunctionType
BF = mybir.dt.bfloat16
F32 = mybir.dt.float32
bf16 = ml_dtypes.bfloat16

# Problem constants (hardcoded; kernel.py must be self-contained)
B, T, CIN, CH, M, NCLS = 2048, 8, 2048, 1024, 32, 21
NCORES = 8
BL = B // NCORES          # 256 batch rows per core
J = NCLS * M              # 672
P = 128
KB = CIN // P             # 16 contraction blocks for kv
OB = 2 * CH // P          # 16 output-channel blocks for kv
KHB = CH // P             # 8 blocks of k/v half
JBS = [P] * 5 + [J - 5 * P]   # j blocks: 5x128 + 32
NB = BL // P              # 2 batch chunks of 128


def build_nc(debug=False, repeat=1):
    nc = bacc.Bacc("TRN2", target_bir_lowering=False, debug=debug,
                   num_devices=NCORES)

    # all inputs are shipped in the exact per-partition SBUF layout so every
    # DMA is a plain [128, N]-contiguous copy (max DMA efficiency)
    xt_e = nc.dram_tensor("xt", [P, KB * BL], BF, kind="ExternalInput")
    wkv_e = nc.dram_tensor("wkv", [OB, P, KB * P], BF, kind="ExternalInput")
    ekn_e = nc.dram_tensor("ekn", [P, KHB * J], BF, kind="ExternalInput")
    amat_e = nc.dram_tensor("amat", [P, 6 * CH], BF, kind="ExternalInput")
    evwb_e = nc.dram_tensor("evwb", [P, J], BF, kind="ExternalInput")
    wout_e = nc.dram_tensor("wout", [P, KB * NCLS], BF, kind="ExternalInput")
    bkv_e = nc.dram_tensor("bkv", [P, OB], F32, kind="ExternalInput")
    bout_e = nc.dram_tensor("bout", [NCLS, 1], F32, kind="ExternalInput")
    ident_e = nc.dram_tensor("ident", [P, P], BF, kind="ExternalInput")
    out_e = nc.dram_tensor("out", [NCLS, BL], F32, kind="ExternalOutput")

    with tile.TileContext(nc) as tc:
        from contextlib import ExitStack
        with ExitStack() as ctx:
            pers = ctx.enter_context(tc.tile_pool(name="pers", bufs=1))
            # ALL psum pools co-resident (1+2+2+1+2 = 8 banks) so no phase
            # ever waits on a pool-scope boundary; Tile interleaves freely.
            pmisc = ctx.enter_context(tc.tile_pool(name="pmisc", bufs=1, space="PSUM"))
            pkv = ctx.enter_context(tc.tile_pool(name="pkv", bufs=2, space="PSUM"))
            pdot = ctx.enter_context(tc.tile_pool(name="pdot", bufs=1, space="PSUM"))
            ptr = ctx.enter_context(tc.tile_pool(name="ptr", bufs=1, space="PSUM"))
            pfe = ctx.enter_context(tc.tile_pool(name="pfe", bufs=1, space="PSUM"))

            # body emitted `repeat` times for delta-timing benchmarks
            # (tags make repeats share SBUF slots; WAR deps serialize them)
            for _rep in range(repeat):
              # ---- SBUF tiles ----
              bkv_s = pers.tile([P, OB], F32, tag="bkv")
              bout_s = pers.tile([NCLS, 1], F32, tag="bout")
              ident_s = pers.tile([P, P], BF, tag="ident")
              evwb_s = pers.tile([P, J], BF, tag="evwb")
              ones_s = pers.tile([P, 1], BF, tag="ones")
              scratch_s = pers.tile([1, 1], F32, tag="scratch")
              xt_s = pers.tile([P, KB * BL], BF, tag="xt")
              wkv_s = pers.tile([P, OB * KB * P], BF, tag="wkv")
              ekn_s = pers.tile([P, KHB * J], BF, tag="ekn")
              a_s = pers.tile([P, 6 * CH], BF, tag="amat")
              wout_s = pers.tile([P, KB * NCLS], BF, tag="wout")
              kt_s = pers.tile([P, KHB * BL], BF, tag="kt")
              ksq_s = pers.tile([P, KHB * BL], BF, tag="ksq")
              hv_s = pers.tile([P, KHB * BL], BF, tag="hv")
              hfe_s = pers.tile([P, KHB * BL], BF, tag="hfe")
              e_s = pers.tile([P, NB * J], BF, tag="e")
              tmp_s = pers.tile([P, J], BF, tag="tmp")
              u_s = pers.tile([P, NB * J], BF, tag="u")
              ut_s = pers.tile([P, 6 * BL], BF, tag="ut")
              rinv_s = pers.tile([P, NB], F32, tag="rinv")
              rs1_s = pers.tile([P, NB], F32, tag="rs1")
              rs2_s = pers.tile([P, NB], F32, tag="rs2")
              magic_s = pers.tile([P, 1], mybir.dt.int32, tag="magic")
              s_s = pers.tile([P, NB * NCLS], F32, tag="s")
              num_s = pers.tile([P, NB * NCLS], F32, tag="num")
              sinv_s = pers.tile([P, NB * NCLS], F32, tag="sinv")
              t_s = pers.tile([P, NB * NCLS], F32, tag="t")
              g_s = pers.tile([P, NB * NCLS], F32, tag="g")
              gg_s = pers.tile([P, NB], F32, tag="gg")
              ginv_s = pers.tile([P, NB], F32, tag="ginv")
              c1_s = pers.tile([P, NB * NCLS], F32, tag="c1")
              c_s = pers.tile([P, NB * NCLS], F32, tag="c")
              out_sb = pers.tile([NCLS, BL], F32, tag="outsb")

              # ---- DMA schedule ----
              # critical path first on the sync (HWDGE) queue: xt quarters,
              # then k-half weight chunks, ekn (dot), v-half chunks with
              # amat/wout slotted before the last two.
              XQ = 4
              qs = KB * BL // XQ
              nc.sync.dma_start(xt_s[:, 0:qs], xt_e.ap()[:, 0:qs])
              # first weight block right after the first xt quarter so PE can
              # start; remaining xt quarters arrive before k-step 4
              nc.sync.dma_start(wkv_s[:, 0:KB * P], wkv_e.ap()[0])
              for q in range(1, XQ):
                  nc.sync.dma_start(xt_s[:, q * qs:(q + 1) * qs],
                                    xt_e.ap()[:, q * qs:(q + 1) * qs])
              nc.gpsimd.dma_start(bkv_s[:], bkv_e.ap())
              nc.gpsimd.dma_start(bout_s[:], bout_e.ap())
              nc.gpsimd.dma_start(ident_s[:], ident_e.ap())
              nc.gpsimd.dma_start(evwb_s[:], evwb_e.ap())
              nc.vector.memset(ones_s[:], 1.0)
              nc.vector.memset(magic_s[:], 0x5f3759df)

              # dummy Exp as the FIRST ACT op pins the exp table set, which
              # also contains Identity/Relu (all ACT fns used here) -> exactly
              # one table load, executed while PE waits on the first weight DMA
              nc.vector.memset(scratch_s[:], 1.0)
              nc.scalar.activation(scratch_s[:], scratch_s[:], AF.Exp)

              # DMA engines are a shared resource: one consumption-ordered
              # stream beats split queues. ekn/amat/wout go last (consumed at
              # ~30/42/55us, all delivered in time).
              for oj in range(1, OB - 4):
                  nc.sync.dma_start(
                      wkv_s[:, oj * KB * P:(oj + 1) * KB * P], wkv_e.ap()[oj])
              nc.sync.dma_start(ekn_s[:], ekn_e.ap())
              for oj in range(OB - 4, OB - 2):
                  nc.sync.dma_start(
                      wkv_s[:, oj * KB * P:(oj + 1) * KB * P], wkv_e.ap()[oj])
              nc.sync.dma_start(a_s[:], amat_e.ap())
              for oj in range(OB - 2, OB):
                  nc.sync.dma_start(
                      wkv_s[:, oj * KB * P:(oj + 1) * KB * P], wkv_e.ap()[oj])
              nc.sync.dma_start(wout_s[:], wout_e.ap())

              # ---- phase 1: kvT = WkvT^T-blocks @ xT; evict k (+bias) / relu(v+bias) ----
              def kv_block(oj):
                  ps = pkv.tile([P, BL], F32, tag="pkv")
                  base = oj * KB * P
                  for i in range(KB):
                      nc.tensor.matmul(ps[:],
                                       wkv_s[:, base + i * P: base + (i + 1) * P],
                                       xt_s[:, i * BL:(i + 1) * BL],
                                       start=(i == 0), stop=(i == KB - 1))
                  if oj < KHB:
                      sl = slice(oj * BL, (oj + 1) * BL)
                      nc.scalar.activation(kt_s[:, sl], ps[:], AF.Identity,
                                           bias=bkv_s[:, oj:oj + 1])
                      nc.vector.tensor_mul(ksq_s[:, sl], kt_s[:, sl], kt_s[:, sl])
                  else:
                      o2 = oj - KHB
                      nc.scalar.activation(hv_s[:, o2 * BL:(o2 + 1) * BL], ps[:],
                                           AF.Relu, bias=bkv_s[:, oj:oj + 1])

              for oj in range(OB - 4):
                  kv_block(oj)

              # ---- phase 2: sumsq via ones-matmul; rinv = rsqrt on DVE ----
              ps2 = pmisc.tile([P, NB], F32, tag="misc")
              for bc in range(NB):
                  for i in range(KHB):
                      nc.tensor.matmul(ps2[:, bc:bc + 1],
                                       ksq_s[:, i * BL + bc * P: i * BL + bc * P + P],
                                       ones_s[:],
                                       start=(i == 0), stop=(i == KHB - 1))
                  # rinv = rsqrt(sumsq) fully on DVE (magic constant + 2
                  # Newton steps, rel err ~4e-6): no ACT table switches
                  sq = rs1_s[:, bc:bc + 1]
                  nc.vector.tensor_copy(sq, ps2[:, bc:bc + 1])
                  y = rinv_s[:, bc:bc + 1]
                  nc.vector.tensor_scalar(
                      y.bitcast(mybir.dt.int32), sq.bitcast(mybir.dt.int32),
                      1, None, op0=mybir.AluOpType.logical_shift_right)
                  nc.vector.tensor_tensor(
                      out=y.bitcast(mybir.dt.int32), in0=magic_s[:],
                      in1=y.bitcast(mybir.dt.int32),
                      op=mybir.AluOpType.subtract)
                  for _ in range(2):
                      t1 = rs2_s[:, bc:bc + 1]
                      nc.vector.tensor_mul(t1, y, y)
                      nc.vector.tensor_mul(t1, t1, sq)
                      nc.vector.tensor_scalar(t1, t1, -0.5, 1.5,
                                              op0=mybir.AluOpType.mult,
                                              op1=mybir.AluOpType.add)
                      nc.vector.tensor_mul(y, y, t1)

              # ---- phase 3 pieces ----
              def dots(bc):
                  psd = pdot.tile([P, J], F32, tag="pdot")
                  for i in range(KHB):
                      lhs = kt_s[:, i * BL + bc * P: i * BL + bc * P + P]
                      nc.tensor.matmul(psd[:, 0:512], lhs,
                                       ekn_s[:, i * J: i * J + 512],
                                       start=(i == 0), stop=(i == KHB - 1))
                      nc.tensor.matmul(psd[:, 512:J], lhs,
                                       ekn_s[:, i * J + 512:(i + 1) * J],
                                       start=(i == 0), stop=(i == KHB - 1))
                  return psd

              def softmax_chain(bc, psd):
                  e_sl = e_s[:, bc * J:(bc + 1) * J]
                  # exp evict in two halves so the next dots() WAR-waits only
                  # half as long on the psd read
                  nc.scalar.activation(e_sl[:, 0:512], psd[:, 0:512], AF.Exp,
                                       scale=rinv_s[:, bc:bc + 1])
                  nc.scalar.activation(e_sl[:, 512:J], psd[:, 512:J], AF.Exp,
                                       scale=rinv_s[:, bc:bc + 1])
                  e3 = e_sl.rearrange("p (n m) -> p n m", m=M)
                  ncls_sl = slice(bc * NCLS, (bc + 1) * NCLS)
                  s2 = s_s[:, ncls_sl]
                  nc.vector.reduce_sum(s2, e3, axis=mybir.AxisListType.X)
                  nc.vector.tensor_mul(tmp_s[:], e_sl, evwb_s[:])
                  nc.vector.reduce_sum(num_s[:, ncls_sl],
                                       tmp_s[:].rearrange("p (n m) -> p n m", m=M),
                                       axis=mybir.AxisListType.X)
                  nc.vector.reciprocal(sinv_s[:, ncls_sl], s2)
                  nc.vector.tensor_mul(t_s[:, ncls_sl], num_s[:, ncls_sl],
                                       sinv_s[:, ncls_sl])
                  nc.scalar.activation(g_s[:, ncls_sl], t_s[:, ncls_sl], AF.Exp)
                  nc.vector.reduce_sum(gg_s[:, bc:bc + 1], g_s[:, ncls_sl],
                                       axis=mybir.AxisListType.X)
                  nc.vector.reciprocal(ginv_s[:, bc:bc + 1], gg_s[:, bc:bc + 1])
                  nc.vector.tensor_mul(c1_s[:, ncls_sl], g_s[:, ncls_sl],
                                       sinv_s[:, ncls_sl])
                  nc.vector.tensor_scalar_mul(c_s[:, ncls_sl], c1_s[:, ncls_sl],
                                              ginv_s[:, bc:bc + 1])
                  c_b = bass.AP(c_s.tensor, c_s[:, ncls_sl].offset,
                                c_s[:, ncls_sl].ap + [[0, M]])
                  u3 = u_s[:, bc * J:(bc + 1) * J].rearrange("p (n m) -> p n m", m=M)
                  nc.vector.tensor_mul(u3, e3, c_b)

              # ---- phase 4+5 per batch chunk: transpose u, then fE matmuls
              # with 8 accumulators packed into two psum banks; the jb0-2
              # matmuls overlap the group-1 eviction copy on DVE ----
              def transpose_fe(bc):
                  def tgroup(g, grp):
                      pst = ptr.tile([P, 3 * P], BF, tag="ptr")
                      for t, jb in enumerate(grp):
                          w = JBS[jb]
                          nc.tensor.transpose(
                              pst[:w, t * P:(t + 1) * P],
                              u_s[:, bc * J + jb * P: bc * J + jb * P + w],
                              ident_s[:])
                      n = sum(1 for jb in grp if JBS[jb] == P)
                      base = ut_s[:, grp[0] * BL + bc * P: grp[0] * BL + bc * P + P]
                      dst = bass.AP(ut_s.tensor, base.offset,
                                    [base.ap[0], [BL, n], base.ap[1]])
                      nc.vector.tensor_copy(
                          dst, pst[:, 0:n * P].rearrange("p (n q) -> p n q", q=P))
                      if n < len(grp):
                          jb = grp[n]
                          w = JBS[jb]
                          nc.vector.tensor_copy(
                              ut_s[:w, jb * BL + bc * P: jb * BL + bc * P + P],
                              pst[:w, n * P:(n + 1) * P])
                  tgroup(0, (0, 1, 2))
                  tgroup(1, (3, 4, 5))
                  if bc < NB - 1:
                      return
                  # ---- phase 5: fE = A^T-blocks @ uT, full batch width;
                  # relu evicts alternate ACT/DVE so neither queue's
                  # per-instruction overhead rate-limits PE ----
                  for oj in range(KHB):
                      acc = pfe.tile([P, BL], F32, tag=f"pfe{oj % 2}")
                      for jb in range(6):
                          w = JBS[jb]
                          nc.tensor.matmul(
                              acc[:],
                              a_s[:w, jb * CH + oj * P: jb * CH + (oj + 1) * P],
                              ut_s[:w, jb * BL:(jb + 1) * BL],
                              start=(jb == 0), stop=(jb == 5))
                      dst = hfe_s[:, oj * BL:(oj + 1) * BL]
                      if oj % 2 == 0:
                          nc.scalar.activation(dst, acc[:], AF.Relu)
                      else:
                          nc.vector.tensor_scalar_max(dst, acc[:], 0.0)

              # kv blocks 12-15 are PE filler under the two softmax chains
              # (dots/exp/DVE chain latency would otherwise idle PE ~7us)
              psd0 = dots(0)
              softmax_chain(0, psd0)
              psd1 = dots(1)
              softmax_chain(1, psd1)
              kv_block(OB - 4)
              transpose_fe(0)
              kv_block(OB - 3)
              transpose_fe(1)
              kv_block(OB - 2)
              kv_block(OB - 1)

              # ---- phase 6: outT = WoutT^T-blocks @ [hv; hfe]; +bout; DMA out ----
              pso = pmisc.tile([NCLS, BL], F32, tag="misc")
              for i in range(KB):
                  h_s = hv_s if i < KHB else hfe_s
                  ii = i % KHB
                  nc.tensor.matmul(pso[:], wout_s[:, i * NCLS:(i + 1) * NCLS],
                                   h_s[:, ii * BL:(ii + 1) * BL],
                                   start=(i == 0), stop=(i == KB - 1))
              nc.vector.tensor_scalar_add(out_sb[:], pso[:], bout_s[:, 0:1])
              nc.sync.dma_start(out_e.ap(), out_sb[:])

    nc.compile()
    return nc


def host_prep(x, static_feat, Wk, bk, Wv, bv, WEk, bEk, WEv, bEv, Ww, bw,
              Wout, bout):
    """Host-side fp32 precompute + per-core input maps."""
    EPS = 1e-8
    f32 = np.float32
    x = np.asarray(x, f32)
    static_feat = np.asarray(static_feat, f32)

    Ek = np.einsum('oc,ncm->nom', np.asarray(WEk, f32), static_feat,
                   optimize=True) + np.asarray(bEk, f32)[None, :, None]
    Ev = np.einsum('oc,ncm->nom', np.asarray(WEv, f32), static_feat,
                   optimize=True) + np.asarray(bEv, f32)[None, :, None]
    Ekn = Ek / np.maximum(np.linalg.norm(Ek, axis=1, keepdims=True), EPS)
    Ekn_mat = Ekn.transpose(1, 0, 2).reshape(CH, J)          # [CH, 672]
    A_mat = Ev.transpose(0, 2, 1).reshape(J, CH)             # [672, CH]
    evwb = np.einsum('nom,o->nm', Ev, np.asarray(Ww, f32)[0]).reshape(J)

    WkvT = np.concatenate([np.asarray(Wk, f32), np.asarray(Wv, f32)], axis=0).T
    bkv = np.concatenate([np.asarray(bk, f32), np.asarray(bv, f32)])
    xT = np.ascontiguousarray(x[:, -1, :].T)                 # [CIN, B]

    # [OB, P, KB*P]: per-o-chunk, per-partition-linear
    wkv_h = np.ascontiguousarray(
        WkvT.reshape(KB, P, OB, P).transpose(2, 1, 0, 3).reshape(
            OB, P, KB * P)).astype(bf16)
    ekn_h = np.ascontiguousarray(
        Ekn_mat.reshape(KHB, P, J).transpose(1, 0, 2).reshape(
            P, KHB * J)).astype(bf16)
    a_pad = np.zeros((6 * P, CH), np.float32)
    a_pad[:J] = A_mat
    amat_h = np.ascontiguousarray(
        a_pad.reshape(6, P, CH).transpose(1, 0, 2).reshape(P, 6 * CH)).astype(bf16)
    evwb_h = np.ascontiguousarray(
        np.broadcast_to(evwb.astype(bf16)[None, :], (P, J)))
    wout_h = np.ascontiguousarray(
        np.asarray(Wout, f32).T.reshape(KB, P, NCLS).transpose(1, 0, 2).reshape(
            P, KB * NCLS)).astype(bf16)
    bkv_h = np.ascontiguousarray(bkv.reshape(OB, P).T)
    bout_h = np.asarray(bout, f32).reshape(NCLS, 1)
    ident_h = np.eye(P, dtype=bf16)

    shared = dict(wkv=wkv_h, ekn=ekn_h, amat=amat_h, evwb=evwb_h,
                  wout=wout_h, bkv=bkv_h, bout=bout_h, ident=ident_h)
    in_maps = []
    for c in range(NCORES):
        xt_h = np.ascontiguousarray(
            xT[:, c * BL:(c + 1) * BL].reshape(KB, P, BL).transpose(1, 0, 2)
            .reshape(P, KB * BL)).astype(bf16)
        in_maps.append(dict(xt=xt_h, **shared))
    return in_maps


_NC_CACHE = {}


def get_nc(debug=False, repeat=1):
    key = (debug, repeat)
    if key not in _NC_CACHE:
        _NC_CACHE[key] = build_nc(debug=debug, repeat=repeat)
    return _NC_CACHE[key]


def kernel(**inputs) -> np.ndarray:
    nc = get_nc()
    in_maps = host_prep(**inputs)
    res = run_bass_kernel_spmd(nc, in_maps, list(range(NCORES)))
    out = np.empty((B, NCLS, 1), dtype=np.float32)
    for c in range(NCORES):
        out[c * BL:(c + 1) * BL, :, 0] = res.results[c]["out"].T
    return out



# revision 8
# speedup vs baseline: 1.3007x; 1.3007x over previous
"""Trainium2 Bass kernel for nn_Colar_static (retrieval_knn).

Sharding: data-parallel over batch B=2048 across 8 NeuronCores (256 rows each).
Static exemplar banks and weights are precomputed/reshaped on host and
replicated to all cores.

v2: fp8e4 DoubleRow matmuls for the x-dependent projections. The cost model
charges DoubleRow fp8 matmuls 0.5 cycles per output row with a doubled
(2x128) contraction per instruction, so:
  - k     = x@Wk.T:  single fp8 pass (the cosine path tolerates ~4% element
            error: cos has ~0.03 absolute scale, softmax logits shift <0.2%)
  - v     = x@Wv.T:  3-pass fp8 residual split (x_hi@W_hi + x_lo@W_hi +
            x_hi@W_lo), ~0.25% error, comparable to bf16
  - dots  = kt.T@Ekn: fp8 DoubleRow (kt evicted to fp8, Ekn pre-quantized)
  - fE, out stay bf16 (their error propagates straight to the output)
This roughly halves DMA bytes (fp8 weights for k, fp8 Ekn) and cuts PE time
~2x, making the single-slot DMA stream (~27.5us) the bottleneck. The DMA
stream is ordered by consumption time so PE/ACT/DVE hide under it.

Per-core pipeline (j = flattened (class, exemplar) = 21*32 = 672):
  1. kT[o,b]  = Wk-blocks^T(fp8 DR) @ xhiT   -> ACT evict +bias -> fp8 kt
  2. ksq      = kt^2 (DVE fp8); sumsq via ones-matmul; rinv = rsqrt (DVE)
  3. dot[b,j] = kt-blocks^T(fp8 DR) @ Ekn
  4. e = exp(rinv*dot) (ACT), blockwise softmax reduces (DVE), u = e*c
  5. uT = PE-transpose(u) (bf16)
  6. v 3-pass fp8 DR accumulate -> ACT relu evict -> hv (bf16)
  7. fE_T[o,b] = A-blocks^T @ uT (bf16), relu evict -> hfe
  8. outT[cls,b] = WoutT-blocks^T @ [hv; hfe] (bf16), interleaved with 6/7
  9. out = psum + bout -> DMA

Host gathers the 8 [21,256] results into [2048, 21, 1].
"""

import numpy as np
import ml_dtypes

import concourse.bass as bass
import concourse.bacc as bacc
import concourse.mybir as mybir
import concourse.tile as tile
from concourse.bass_utils import run_bass_kernel_spmd

AF = mybir.ActivationFunctionType
BF = mybir.dt.bfloat16
F8 = mybir.dt.float8e4
F32 = mybir.dt.float32
DR = mybir.MatmulPerfMode.DoubleRow
bf16 = ml_dtypes.bfloat16
f8 = ml_dtypes.float8_e4m3

# Problem constants (hardcoded; kernel.py must be self-contained)
B, T, CIN, CH, M, NCLS = 2048, 8, 2048, 1024, 32, 21
NCORES = 8
BL = B // NCORES          # 256 batch rows per core
J = NCLS * M              # 672
P = 128
DK = CIN // (2 * P)       # 8 DoubleRow contraction steps for K=2048
D2 = CH // (2 * P)        # 4 DoubleRow steps for K=1024 (dots)
KB = CIN // P             # 16 bf16 contraction blocks (out matmul)
KHB = CH // P             # 8 blocks of k/v half
JBS = [P] * 5 + [J - 5 * P]   # j blocks: 5x128 + 32
NB = BL // P              # 2 batch chunks of 128
JC = [(0, 256), (256, 256), (512, J - 512)]  # dots moving-dim chunks
WS = 64.0                 # fp8 weight pre-scale: keeps W and its residual out
                          # of e4m3's subnormal range; undone at ACT eviction


def build_nc(debug=False, repeat=1):
    nc = bacc.Bacc("TRN2", target_bir_lowering=False, debug=debug,
                   num_devices=NCORES)

    # inputs shipped in exact per-partition SBUF layout: every DMA is a
    # plain [128, N]-contiguous copy
    xhi_e = nc.dram_tensor("xhi", [P, DK * 2 * BL], F8, kind="ExternalInput")
    xlo_e = nc.dram_tensor("xlo", [P, DK * 2 * BL], F8, kind="ExternalInput")
    wk_e = nc.dram_tensor("wk", [KHB, P, DK * 2 * P], F8, kind="ExternalInput")
    wvh_e = nc.dram_tensor("wvh", [KHB, P, DK * 2 * P], F8, kind="ExternalInput")
    wvl_e = nc.dram_tensor("wvl", [KHB, P, DK * 2 * P], F8, kind="ExternalInput")
    ekn_e = nc.dram_tensor("ekn", [P, D2 * 2 * J], F8, kind="ExternalInput")
    amat_e = nc.dram_tensor("amat", [KHB, P, 6 * P], BF, kind="ExternalInput")
    evwb_e = nc.dram_tensor("evwb", [P, J], BF, kind="ExternalInput")
    wout_e = nc.dram_tensor("wout", [P, KB * NCLS], BF, kind="ExternalInput")
    bkv_e = nc.dram_tensor("bkv", [P, 2 * KHB], F32, kind="ExternalInput")
    bout_e = nc.dram_tensor("bout", [NCLS, 1], F32, kind="ExternalInput")
    ident_e = nc.dram_tensor("ident", [P, P], BF, kind="ExternalInput")
    out_e = nc.dram_tensor("out", [NCLS, BL], F32, kind="ExternalOutput")

    with tile.TileContext(nc) as tc:
        from contextlib import ExitStack
        with ExitStack() as ctx:
            pers = ctx.enter_context(tc.tile_pool(name="pers", bufs=1))
            # psum budget (bank-per-tile): pmisc 1 + pkv 2 + pdot 2 + ptr 1
            # + pfe 2 = 8 banks, all co-resident.
            pmisc = ctx.enter_context(tc.tile_pool(name="pmisc", bufs=1, space="PSUM"))
            pkv = ctx.enter_context(tc.tile_pool(name="pkv", bufs=2, space="PSUM"))
            pdot = ctx.enter_context(tc.tile_pool(name="pdot", bufs=1, space="PSUM"))
            ptr = ctx.enter_context(tc.tile_pool(name="ptr", bufs=1, space="PSUM"))
            pfe = ctx.enter_context(tc.tile_pool(name="pfe", bufs=1, space="PSUM"))

            for _rep in range(repeat):
              # ---- SBUF tiles ----
              bkv_s = pers.tile([P, 2 * KHB], F32, tag="bkv")
              bout_s = pers.tile([NCLS, 1], F32, tag="bout")
              ident_s = pers.tile([P, P], BF, tag="ident")
              evwb_s = pers.tile([P, J], BF, tag="evwb")
              ones_s = pers.tile([P, 1], F8, tag="ones")
              scratch_s = pers.tile([1, 1], F32, tag="scratch")
              xhi_s = pers.tile([P, DK * 2 * BL], F8, tag="xhi")
              xlo_s = pers.tile([P, DK * 2 * BL], F8, tag="xlo")
              wk_s = pers.tile([P, KHB * DK * 2 * P], F8, tag="wk")
              wvh_s = pers.tile([P, KHB * DK * 2 * P], F8, tag="wvh")
              wvl_s = pers.tile([P, KHB * DK * 2 * P], F8, tag="wvl")
              ekn_s = pers.tile([P, D2 * 2 * J], F8, tag="ekn")
              a_s = pers.tile([P, KHB * 6 * P], BF, tag="amat")
              wout_s = pers.tile([P, KB * NCLS], BF, tag="wout")
              kt_s = pers.tile([P, KHB * BL], F8, tag="kt")
              ksq_s = pers.tile([P, KHB * BL], F8, tag="ksq")
              hv_s = pers.tile([P, KHB * BL], BF, tag="hv")
              hfe_s = pers.tile([P, KHB * BL], BF, tag="hfe")
              e_s = pers.tile([P, NB * J], BF, tag="e")
              tmp_s = pers.tile([P, J], BF, tag="tmp")
              u_s = pers.tile([P, NB * J], BF, tag="u")
              ut_s = pers.tile([P, 6 * BL], BF, tag="ut")
              rinv_s = pers.tile([P, NB], F32, tag="rinv")
              rs1_s = pers.tile([P, NB], F32, tag="rs1")
              rs2_s = pers.tile([P, NB], F32, tag="rs2")
              magic_s = pers.tile([P, 1], mybir.dt.int32, tag="magic")
              s_s = pers.tile([P, NB * NCLS], F32, tag="s")
              num_s = pers.tile([P, NB * NCLS], F32, tag="num")
              sinv_s = pers.tile([P, NB * NCLS], F32, tag="sinv")
              t_s = pers.tile([P, NB * NCLS], F32, tag="t")
              g_s = pers.tile([P, NB * NCLS], F32, tag="g")
              gg_s = pers.tile([P, NB], F32, tag="gg")
              ginv_s = pers.tile([P, NB], F32, tag="ginv")
              c1_s = pers.tile([P, NB * NCLS], F32, tag="c1")
              c_s = pers.tile([P, NB * NCLS], F32, tag="c")
              out_sb = pers.tile([NCLS, BL], F32, tag="outsb")

              CW = DK * 2 * P   # 2048 fp8 weight elems per (block, all dsteps)

              # ---- DMA schedule: one consumption-ordered stream on the sync
              # (HWDGE) queue; tiny tensors on the gpsimd (SWDGE) queue.
              nc.sync.dma_start(xhi_s[:], xhi_e.ap())
              for oj in range(6):
                  nc.sync.dma_start(wk_s[:, oj * CW:(oj + 1) * CW], wk_e.ap()[oj])
              nc.gpsimd.dma_start(bkv_s[:], bkv_e.ap())
              nc.gpsimd.dma_start(bout_s[:], bout_e.ap())
              nc.gpsimd.dma_start(ident_s[:], ident_e.ap())
              nc.gpsimd.dma_start(evwb_s[:], evwb_e.ap())
              nc.vector.memset(ones_s[:], 1.0)
              nc.vector.memset(magic_s[:], 0x5f3759df)

              # dummy Exp as the FIRST ACT op pins the exp table set (contains
              # Identity/Relu too) -> exactly one table load, during DMA wait
              nc.vector.memset(scratch_s[:], 1.0)
              nc.scalar.activation(scratch_s[:], scratch_s[:], AF.Exp)

              nc.sync.dma_start(ekn_s[:], ekn_e.ap())
              for oj in range(6, KHB):
                  nc.sync.dma_start(wk_s[:, oj * CW:(oj + 1) * CW], wk_e.ap()[oj])
              nc.sync.dma_start(xlo_s[:], xlo_e.ap())
              # v-phase stream: [wvh_oj, (amat), wvl_oj] per oj; amat chunks
              # slot in from oj=2 so fE_oj can chase; amat6/7 + wout go before
              # the final wvl so the tail is only v-pass3 work.
              AMAT_AT = {2: 0, 3: 1, 4: 2, 5: 3, 6: 4, 7: 5}
              for oj in range(KHB):
                  nc.sync.dma_start(wvh_s[:, oj * CW:(oj + 1) * CW], wvh_e.ap()[oj])
                  if oj in AMAT_AT:
                      aj = AMAT_AT[oj]
                      nc.sync.dma_start(a_s[:, aj * 6 * P:(aj + 1) * 6 * P],
                                        amat_e.ap()[aj])
                  if oj == KHB - 1:
                      nc.sync.dma_start(a_s[:, 6 * 6 * P:7 * 6 * P], amat_e.ap()[6])
                      nc.sync.dma_start(a_s[:, 7 * 6 * P:8 * 6 * P], amat_e.ap()[7])
                      nc.sync.dma_start(wout_s[:], wout_e.ap())
                  nc.sync.dma_start(wvl_s[:, oj * CW:(oj + 1) * CW], wvl_e.ap()[oj])

              xhi3 = xhi_s[:].rearrange("p (d two b) -> p d two b", d=DK, two=2)
              xlo3 = xlo_s[:].rearrange("p (d two b) -> p d two b", d=DK, two=2)

              def wblk(ws, oj, d):
                  base = oj * CW + d * 2 * P
                  return ws[:, base:base + 2 * P].rearrange(
                      "p (two m) -> p two m", two=2)

              # ---- phase 1: kT blocks (fp8 DR), evict +bias to fp8 kt,
              # square on DVE ----
              def k_block(oj):
                  ps = pkv.tile([P, BL], F32, tag="pkv")
                  for d in range(DK):
                      nc.tensor.matmul(ps[:], wblk(wk_s, oj, d), xhi3[:, d],
                                       start=(d == 0), stop=(d == DK - 1),
                                       perf_mode=DR)
                  sl = slice(oj * BL, (oj + 1) * BL)
                  nc.scalar.activation(kt_s[:, sl], ps[:], AF.Identity,
                                       bias=bkv_s[:, oj:oj + 1], scale=1.0 / WS)
                  nc.vector.tensor_mul(ksq_s[:, sl], kt_s[:, sl], kt_s[:, sl])

              for oj in range(KHB):
                  k_block(oj)

              # ---- phase 3 pieces (dots fp8 DR) ----
              kt3 = kt_s[:].rearrange("p (o b) -> p o b", o=KHB)
              ekn4 = ekn_s[:].rearrange("p (d two j) -> p d two j", d=D2, two=2)

              def dots(bc):
                  psd = pdot.tile([P, J], F32, tag="pdot")
                  for d in range(D2):
                      lhs = kt3[:, 2 * d:2 * d + 2, bc * P:(bc + 1) * P]
                      for (j0, nj) in JC:
                          nc.tensor.matmul(psd[:, j0:j0 + nj], lhs,
                                           ekn4[:, d, :, j0:j0 + nj],
                                           start=(d == 0), stop=(d == D2 - 1),
                                           perf_mode=DR)
                  return psd

              def softmax_chain(bc, psd):
                  e_sl = e_s[:, bc * J:(bc + 1) * J]
                  nc.scalar.activation(e_sl[:, 0:512], psd[:, 0:512], AF.Exp,
                                       scale=rinv_s[:, bc:bc + 1])
                  nc.scalar.activation(e_sl[:, 512:J], psd[:, 512:J], AF.Exp,
                                       scale=rinv_s[:, bc:bc + 1])
                  e3 = e_sl.rearrange("p (n m) -> p n m", m=M)
                  ncls_sl = slice(bc * NCLS, (bc + 1) * NCLS)
                  s2 = s_s[:, ncls_sl]
                  nc.vector.reduce_sum(s2, e3, axis=mybir.AxisListType.X)
                  nc.vector.tensor_mul(tmp_s[:], e_sl, evwb_s[:])
                  nc.vector.reduce_sum(num_s[:, ncls_sl],
                                       tmp_s[:].rearrange("p (n m) -> p n m", m=M),
                                       axis=mybir.AxisListType.X)
                  nc.vector.reciprocal(sinv_s[:, ncls_sl], s2)
                  nc.vector.tensor_mul(t_s[:, ncls_sl], num_s[:, ncls_sl],
                                       sinv_s[:, ncls_sl])
                  nc.scalar.activation(g_s[:, ncls_sl], t_s[:, ncls_sl], AF.Exp)
                  nc.vector.reduce_sum(gg_s[:, bc:bc + 1], g_s[:, ncls_sl],
                                       axis=mybir.AxisListType.X)
                  nc.vector.reciprocal(ginv_s[:, bc:bc + 1], gg_s[:, bc:bc + 1])
                  nc.vector.tensor_mul(c1_s[:, ncls_sl], g_s[:, ncls_sl],
                                       sinv_s[:, ncls_sl])
                  nc.vector.tensor_scalar_mul(c_s[:, ncls_sl], c1_s[:, ncls_sl],
                                              ginv_s[:, bc:bc + 1])
                  c_b = bass.AP(c_s.tensor, c_s[:, ncls_sl].offset,
                                c_s[:, ncls_sl].ap + [[0, M]])
                  u3 = u_s[:, bc * J:(bc + 1) * J].rearrange("p (n m) -> p n m", m=M)
                  nc.vector.tensor_mul(u3, e3, c_b)

              # ---- phase 2: sumsq via fp8 ones-matmul; rinv = rsqrt on DVE ----
              def sumsq():
                  ps2 = pmisc.tile([P, NB], F32, tag="misc")
                  for bc in range(NB):
                      for i in range(KHB):
                          nc.tensor.matmul(ps2[:, bc:bc + 1],
                                           ksq_s[:, i * BL + bc * P: i * BL + bc * P + P],
                                           ones_s[:],
                                           start=(i == 0), stop=(i == KHB - 1))
                      # rinv = rsqrt(sumsq) fully on DVE (magic + 2 Newton
                      # steps, rel err ~4e-6): no ACT table switches
                      sq = rs1_s[:, bc:bc + 1]
                      nc.vector.tensor_copy(sq, ps2[:, bc:bc + 1])
                      y = rinv_s[:, bc:bc + 1]
                      nc.vector.tensor_scalar(
                          y.bitcast(mybir.dt.int32), sq.bitcast(mybir.dt.int32),
                          1, None, op0=mybir.AluOpType.logical_shift_right)
                      nc.vector.tensor_tensor(
                          out=y.bitcast(mybir.dt.int32), in0=magic_s[:],
                          in1=y.bitcast(mybir.dt.int32),
                          op=mybir.AluOpType.subtract)
                      for _ in range(2):
                          t1 = rs2_s[:, bc:bc + 1]
                          nc.vector.tensor_mul(t1, y, y)
                          nc.vector.tensor_mul(t1, t1, sq)
                          nc.vector.tensor_scalar(t1, t1, -0.5, 1.5,
                                                  op0=mybir.AluOpType.mult,
                                                  op1=mybir.AluOpType.add)
                          nc.vector.tensor_mul(y, y, t1)

              # ---- transpose of u into ut (bf16, PE + DVE evictions) ----
              def transpose_u(bc):
                  def tgroup(grp):
                      pst = ptr.tile([P, 3 * P], BF, tag="ptr")
                      for t, jb in enumerate(grp):
                          w = JBS[jb]
                          nc.tensor.transpose(
                              pst[:w, t * P:(t + 1) * P],
                              u_s[:, bc * J + jb * P: bc * J + jb * P + w],
                              ident_s[:])
                      n = sum(1 for jb in grp if JBS[jb] == P)
                      base = ut_s[:, grp[0] * BL + bc * P: grp[0] * BL + bc * P + P]
                      dst = bass.AP(ut_s.tensor, base.offset,
                                    [base.ap[0], [BL, n], base.ap[1]])
                      nc.vector.tensor_copy(
                          dst, pst[:, 0:n * P].rearrange("p (n q) -> p n q", q=P))
                      if n < len(grp):
                          jb = grp[n]
                          w = JBS[jb]
                          nc.vector.tensor_copy(
                              ut_s[:w, jb * BL + bc * P: jb * BL + bc * P + P],
                              pst[:w, n * P:(n + 1) * P])
                  tgroup((0, 1, 2))
                  tgroup((3, 4, 5))

              # ---- v passes (fp8 DR) ----
              v_ps = {}

              def v12_block(oj):
                  ps = pkv.tile([P, BL], F32, tag="pkv")
                  v_ps[oj] = ps
                  for d in range(DK):
                      nc.tensor.matmul(ps[:], wblk(wvh_s, oj, d), xhi3[:, d],
                                       start=(d == 0), stop=False, perf_mode=DR)
                      nc.tensor.matmul(ps[:], wblk(wvh_s, oj, d), xlo3[:, d],
                                       start=False, stop=False, perf_mode=DR)

              def v3_block(oj):
                  ps = v_ps.pop(oj)
                  for d in range(DK):
                      nc.tensor.matmul(ps[:], wblk(wvl_s, oj, d), xhi3[:, d],
                                       start=False, stop=(d == DK - 1),
                                       perf_mode=DR)
                  nc.scalar.activation(hv_s[:, oj * BL:(oj + 1) * BL], ps[:],
                                       AF.Relu, bias=bkv_s[:, KHB + oj:KHB + oj + 1],
                                       scale=1.0 / WS)

              # ---- fE blocks (bf16) ----
              def fe_block(oj):
                  acc = pfe.tile([P, BL], F32, tag=f"pfe{oj % 2}")
                  for jb in range(6):
                      w = JBS[jb]
                      nc.tensor.matmul(
                          acc[:],
                          a_s[:w, oj * 6 * P + jb * P: oj * 6 * P + (jb + 1) * P],
                          ut_s[:w, jb * BL:(jb + 1) * BL],
                          start=(jb == 0), stop=(jb == 5))
                  dst = hfe_s[:, oj * BL:(oj + 1) * BL]
                  if oj % 2 == 0:
                      nc.scalar.activation(dst, acc[:], AF.Relu)
                  else:
                      nc.vector.tensor_scalar_max(dst, acc[:], 0.0)

              # ---- out accumulation (bf16), interleaved ----
              pso = pmisc.tile([NCLS, BL], F32, tag="misc")
              out_emitted = [0]

              def out_step(i, last=False):
                  h_s = hv_s if i < KHB else hfe_s
                  ii = i % KHB
                  nc.tensor.matmul(pso[:], wout_s[:, i * NCLS:(i + 1) * NCLS],
                                   h_s[:, ii * BL:(ii + 1) * BL],
                                   start=(out_emitted[0] == 0), stop=last)
                  out_emitted[0] += 1

              # ---- PE-ordered program ----
              # dots0 -> sumsq -> dots1 avoids a WAR deadlock: dots1 reuses
              # the pdot slot so it waits on exp0, which waits on rinv, which
              # needs the sumsq matmuls -- those must precede dots1 on PE.
              psd0 = dots(0)
              sumsq()
              psd1 = dots(1)
              softmax_chain(0, psd0)
              softmax_chain(1, psd1)
              v12_block(0)
              v3_block(0)
              v12_block(1)
              v3_block(1)
              transpose_u(0)
              transpose_u(1)
              for oj in range(2, KHB - 1):
                  v12_block(oj)
                  fe_block(oj - 2)
                  v3_block(oj)
                  # lag out steps: their h inputs were evicted earlier
                  out_step(oj - 2)                   # hv_{oj-2}
                  if oj >= 3:
                      out_step(KHB + oj - 3)         # hfe_{oj-3}
              # tail: fe5/6/7 run while wvl7 (the last transfer) is in
              # flight; only v3_7 + out steps remain after it lands.
              v12_block(KHB - 1)
              fe_block(5)
              out_step(5)                            # hv5
              out_step(KHB + 4)                      # hfe4
              fe_block(6)
              out_step(KHB + 5)                      # hfe5
              fe_block(7)
              out_step(KHB + 6)                      # hfe6
              v3_block(KHB - 1)
              out_step(6)                            # hv6
              out_step(KHB + 7)                      # hfe7
              out_step(7, last=True)                 # hv7

              nc.vector.tensor_scalar_add(out_sb[:], pso[:], bout_s[:, 0:1])
              nc.sync.dma_start(out_e.ap(), out_sb[:])

    nc.compile()
    return nc


def host_prep(x, static_feat, Wk, bk, Wv, bv, WEk, bEk, WEv, bEv, Ww, bw,
              Wout, bout):
    """Host-side fp32 precompute + per-core input maps."""
    EPS = 1e-8
    f32 = np.float32
    x = np.asarray(x, f32)
    static_feat = np.asarray(static_feat, f32)

    Ek = np.einsum('oc,ncm->nom', np.asarray(WEk, f32), static_feat,
                   optimize=True) + np.asarray(bEk, f32)[None, :, None]
    Ev = np.einsum('oc,ncm->nom', np.asarray(WEv, f32), static_feat,
                   optimize=True) + np.asarray(bEv, f32)[None, :, None]
    Ekn = Ek / np.maximum(np.linalg.norm(Ek, axis=1, keepdims=True), EPS)
    Ekn_mat = Ekn.transpose(1, 0, 2).reshape(CH, J)          # [CH, 672]
    A_mat = Ev.transpose(0, 2, 1).reshape(J, CH)             # [672, CH]
    evwb = np.einsum('nom,o->nm', Ev, np.asarray(Ww, f32)[0]).reshape(J)

    xT = np.ascontiguousarray(x[:, -1, :].T)                 # [CIN, B]
    xT_hi = xT.astype(f8)
    xT_lo = (xT - xT_hi.astype(f32)).astype(f8)

    def wsplit(W):
        # W [CH, CIN] -> WT*WS [CIN, CH] -> hi/lo fp8 (scaled out of
        # e4m3 subnormals; eviction divides by WS)
        WT = np.asarray(W, f32).T * WS
        hi = WT.astype(f8)
        lo = (WT - hi.astype(f32)).astype(f8)
        return hi, lo

    wk_hi, _ = wsplit(Wk)
    wv_hi, wv_lo = wsplit(Wv)

    def wlayout(WT8):
        # [CIN, CH] fp8 -> [KHB(oj), P, DK*2*P]: per (oj, d): [p, 2, 128m]
        a = WT8.reshape(DK, 2, P, KHB, P)        # (d, two, p, oj, m)
        return np.ascontiguousarray(
            a.transpose(3, 2, 0, 1, 4).reshape(KHB, P, DK * 2 * P))

    wk_h = wlayout(wk_hi)
    wvh_h = wlayout(wv_hi)
    wvl_h = wlayout(wv_lo)

    # ekn fp8: [CH, J] -> [P, D2*2*J]
    ekn8 = Ekn_mat.astype(f8)
    ekn_h = np.ascontiguousarray(
        ekn8.reshape(D2, 2, P, J).transpose(2, 0, 1, 3).reshape(P, D2 * 2 * J))

    # amat bf16: padded [768, CH] -> [KHB(oj), P, 6*P]
    a_pad = np.zeros((6 * P, CH), np.float32)
    a_pad[:J] = A_mat
    amat_h = np.ascontiguousarray(
        a_pad.reshape(6, P, KHB, P).transpose(2, 1, 0, 3).reshape(
            KHB, P, 6 * P)).astype(bf16)

    evwb_h = np.ascontiguousarray(
        np.broadcast_to(evwb.astype(bf16)[None, :], (P, J)))
    wout_h = np.ascontiguousarray(
        np.asarray(Wout, f32).T.reshape(KB, P, NCLS).transpose(1, 0, 2).reshape(
            P, KB * NCLS)).astype(bf16)
    bkv = np.concatenate([np.asarray(bk, f32), np.asarray(bv, f32)])
    bkv_h = np.ascontiguousarray(bkv.reshape(2 * KHB, P).T)
    bout_h = np.asarray(bout, f32).reshape(NCLS, 1)
    ident_h = np.eye(P, dtype=bf16)

    shared = dict(wk=wk_h, wvh=wvh_h, wvl=wvl_h, ekn=ekn_h, amat=amat_h,
                  evwb=evwb_h, wout=wout_h, bkv=bkv_h, bout=bout_h,
                  ident=ident_h)
    in_maps = []
    for c in range(NCORES):
        sl = slice(c * BL, (c + 1) * BL)

        def xlayout(x8):
            # [CIN, BL] fp8 -> [P, DK*2*BL]: per d: [p, 2, BL]
            a = x8[:, sl].reshape(DK, 2, P, BL)
            return np.ascontiguousarray(
                a.transpose(2, 0, 1, 3).reshape(P, DK * 2 * BL))

        in_maps.append(dict(xhi=xlayout(xT_hi), xlo=xlayout(xT_lo), **shared))
    return in_maps


_NC_CACHE = {}


def get_nc(debug=False, repeat=1):
    key = (debug, repeat)
    if key not in _NC_CACHE:
        _NC_CACHE[key] = build_nc(debug=debug, repeat=repeat)
    return _NC_CACHE[key]


def kernel(**inputs) -> np.ndarray:
    nc = get_nc()
    in_maps = host_prep(**inputs)
    res = run_bass_kernel_spmd(nc, in_maps, list(range(NCORES)))
    out = np.empty((B, NCLS, 1), dtype=np.float32)
    for c in range(NCORES):
        out[c * BL:(c + 1) * BL, :, 0] = res.results[c]["out"].T
    return out


# revision 12
# speedup vs baseline: 1.3018x; 1.0009x over previous
"""Trainium2 Bass kernel for nn_Colar_static (retrieval_knn).

Sharding: data-parallel over batch B=2048 across 8 NeuronCores (256 rows each).
Static exemplar banks and weights are precomputed/reshaped on host and
replicated to all cores.

v2: fp8e4 DoubleRow matmuls for the x-dependent projections. The cost model
charges DoubleRow fp8 matmuls 0.5 cycles per output row with a doubled
(2x128) contraction per instruction, so:
  - k     = x@Wk.T:  single fp8 pass (the cosine path tolerates ~4% element
            error: cos has ~0.03 absolute scale, softmax logits shift <0.2%)
  - v     = x@Wv.T:  3-pass fp8 residual split (x_hi@W_hi + x_lo@W_hi +
            x_hi@W_lo), ~0.25% error, comparable to bf16
  - dots  = kt.T@Ekn: fp8 DoubleRow (kt evicted to fp8, Ekn pre-quantized)
  - fE, out stay bf16 (their error propagates straight to the output)
This roughly halves DMA bytes (fp8 weights for k, fp8 Ekn) and cuts PE time
~2x, making the single-slot DMA stream (~27.5us) the bottleneck. The DMA
stream is ordered by consumption time so PE/ACT/DVE hide under it.

Per-core pipeline (j = flattened (class, exemplar) = 21*32 = 672):
  1. kT[o,b]  = Wk-blocks^T(fp8 DR) @ xhiT   -> ACT evict +bias -> fp8 kt
  2. ksq      = kt^2 (DVE fp8); sumsq via ones-matmul; rinv = rsqrt (DVE)
  3. dot[b,j] = kt-blocks^T(fp8 DR) @ Ekn
  4. e = exp(rinv*dot) (ACT), blockwise softmax reduces (DVE), u = e*c
  5. uT = PE-transpose(u) (bf16)
  6. v 3-pass fp8 DR accumulate -> ACT relu evict -> hv (bf16)
  7. fE_T[o,b] = A-blocks^T @ uT (bf16), relu evict -> hfe
  8. outT[cls,b] = WoutT-blocks^T @ [hv; hfe] (bf16), interleaved with 6/7
  9. out = psum + bout -> DMA

Host gathers the 8 [21,256] results into [2048, 21, 1].
"""

import numpy as np
import ml_dtypes

import concourse.bass as bass
import concourse.bacc as bacc
import concourse.mybir as mybir
import concourse.tile as tile
from concourse.bass_utils import run_bass_kernel_spmd

AF = mybir.ActivationFunctionType
BF = mybir.dt.bfloat16
F8 = mybir.dt.float8e4
F32 = mybir.dt.float32
DR = mybir.MatmulPerfMode.DoubleRow
bf16 = ml_dtypes.bfloat16
f8 = ml_dtypes.float8_e4m3

# Problem constants (hardcoded; kernel.py must be self-contained)
B, T, CIN, CH, M, NCLS = 2048, 8, 2048, 1024, 32, 21
NCORES = 8
BL = B // NCORES          # 256 batch rows per core
J = NCLS * M              # 672
P = 128
DK = CIN // (2 * P)       # 8 DoubleRow contraction steps for K=2048
D2 = CH // (2 * P)        # 4 DoubleRow steps for K=1024 (dots)
KB = CIN // P             # 16 bf16 contraction blocks (out matmul)
KHB = CH // P             # 8 blocks of k/v half
JBS = [P] * 5 + [J - 5 * P]   # j blocks: 5x128 + 32
NB = BL // P              # 2 batch chunks of 128
JC = [(0, 256), (256, 256), (512, J - 512)]  # dots moving-dim chunks
WS = 64.0                 # fp8 weight pre-scale: keeps W and its residual out
                          # of e4m3's subnormal range; undone at ACT eviction


def build_nc(debug=False, repeat=1):
    nc = bacc.Bacc("TRN2", target_bir_lowering=False, debug=debug,
                   num_devices=NCORES)

    # inputs shipped in exact per-partition SBUF layout: every DMA is a
    # plain [128, N]-contiguous copy
    xhi_e = nc.dram_tensor("xhi", [P, DK * 2 * BL], F8, kind="ExternalInput")
    xlo_e = nc.dram_tensor("xlo", [P, DK * 2 * BL], F8, kind="ExternalInput")
    wk_e = nc.dram_tensor("wk", [KHB, P, DK * 2 * P], F8, kind="ExternalInput")
    wvh_e = nc.dram_tensor("wvh", [KHB, P, DK * 2 * P], F8, kind="ExternalInput")
    wvl_e = nc.dram_tensor("wvl", [KHB, P, DK * 2 * P], F8, kind="ExternalInput")
    ekn_e = nc.dram_tensor("ekn", [P, D2 * 2 * J], F8, kind="ExternalInput")
    amat_e = nc.dram_tensor("amat", [KHB, P, 6 * P], BF, kind="ExternalInput")
    evwb_e = nc.dram_tensor("evwb", [P, J], BF, kind="ExternalInput")
    wout_e = nc.dram_tensor("wout", [P, KB * NCLS], BF, kind="ExternalInput")
    bkv_e = nc.dram_tensor("bkv", [P, 2 * KHB], F32, kind="ExternalInput")
    bout_e = nc.dram_tensor("bout", [NCLS, 1], F32, kind="ExternalInput")
    ident_e = nc.dram_tensor("ident", [P, P], BF, kind="ExternalInput")
    out_e = nc.dram_tensor("out", [NCLS, BL], F32, kind="ExternalOutput")

    with tile.TileContext(nc) as tc:
        from contextlib import ExitStack
        with ExitStack() as ctx:
            pers = ctx.enter_context(tc.tile_pool(name="pers", bufs=1))
            # psum budget (bank-per-tile): pmisc 1 + pkv 2 + pdot 2 + ptr 1
            # + pfe 2 = 8 banks, all co-resident.
            pmisc = ctx.enter_context(tc.tile_pool(name="pmisc", bufs=1, space="PSUM"))
            pkv = ctx.enter_context(tc.tile_pool(name="pkv", bufs=2, space="PSUM"))
            pdot = ctx.enter_context(tc.tile_pool(name="pdot", bufs=1, space="PSUM"))
            ptr = ctx.enter_context(tc.tile_pool(name="ptr", bufs=1, space="PSUM"))
            pfe = ctx.enter_context(tc.tile_pool(name="pfe", bufs=1, space="PSUM"))

            for _rep in range(repeat):
              # ---- SBUF tiles ----
              bkv_s = pers.tile([P, 2 * KHB], F32, tag="bkv")
              bout_s = pers.tile([NCLS, 1], F32, tag="bout")
              ident_s = pers.tile([P, P], BF, tag="ident")
              evwb_s = pers.tile([P, J], BF, tag="evwb")
              ones_s = pers.tile([P, 1], F8, tag="ones")
              scratch_s = pers.tile([1, 1], F32, tag="scratch")
              xhi_s = pers.tile([P, DK * 2 * BL], F8, tag="xhi")
              xlo_s = pers.tile([P, DK * 2 * BL], F8, tag="xlo")
              wk_s = pers.tile([P, KHB * DK * 2 * P], F8, tag="wk")
              wvh_s = pers.tile([P, KHB * DK * 2 * P], F8, tag="wvh")
              wvl_s = pers.tile([P, KHB * DK * 2 * P], F8, tag="wvl")
              ekn_s = pers.tile([P, D2 * 2 * J], F8, tag="ekn")
              a_s = pers.tile([P, KHB * 6 * P], BF, tag="amat")
              wout_s = pers.tile([P, KB * NCLS], BF, tag="wout")
              kt_s = pers.tile([P, KHB * BL], F8, tag="kt")
              ksq_s = pers.tile([P, KHB * BL], F8, tag="ksq")
              hv_s = pers.tile([P, KHB * BL], BF, tag="hv")
              hfe_s = pers.tile([P, KHB * BL], BF, tag="hfe")
              e_s = pers.tile([P, NB * J], BF, tag="e")
              tmp_s = pers.tile([P, J], BF, tag="tmp")
              u_s = pers.tile([P, NB * J], BF, tag="u")
              ut_s = pers.tile([P, 6 * BL], BF, tag="ut")
              rinv_s = pers.tile([P, NB], F32, tag="rinv")
              rs1_s = pers.tile([P, NB], F32, tag="rs1")
              rs2_s = pers.tile([P, NB], F32, tag="rs2")
              magic_s = pers.tile([P, 1], mybir.dt.int32, tag="magic")
              s_s = pers.tile([P, NB * NCLS], F32, tag="s")
              num_s = pers.tile([P, NB * NCLS], F32, tag="num")
              sinv_s = pers.tile([P, NB * NCLS], F32, tag="sinv")
              t_s = pers.tile([P, NB * NCLS], F32, tag="t")
              g_s = pers.tile([P, NB * NCLS], F32, tag="g")
              gg_s = pers.tile([P, NB], F32, tag="gg")
              ginv_s = pers.tile([P, NB], F32, tag="ginv")
              c1_s = pers.tile([P, NB * NCLS], F32, tag="c1")
              c_s = pers.tile([P, NB * NCLS], F32, tag="c")
              out_sb = pers.tile([NCLS, BL], F32, tag="outsb")

              CW = DK * 2 * P   # 2048 fp8 weight elems per (block, all dsteps)

              # ---- DMA schedule: one consumption-ordered stream on the sync
              # (HWDGE) queue; tiny tensors on the gpsimd (SWDGE) queue.
              nc.sync.dma_start(xhi_s[:], xhi_e.ap())
              for oj in range(6):
                  nc.sync.dma_start(wk_s[:, oj * CW:(oj + 1) * CW], wk_e.ap()[oj])
              nc.gpsimd.dma_start(bkv_s[:], bkv_e.ap())
              nc.gpsimd.dma_start(bout_s[:], bout_e.ap())
              nc.gpsimd.dma_start(ident_s[:], ident_e.ap())
              nc.gpsimd.dma_start(evwb_s[:], evwb_e.ap())
              nc.vector.memset(ones_s[:], 1.0)
              nc.vector.memset(magic_s[:], 0x5f3759df)

              # dummy Exp as the FIRST ACT op pins the exp table set (contains
              # Identity/Relu too) -> exactly one table load, during DMA wait
              nc.vector.memset(scratch_s[:], 1.0)
              nc.scalar.activation(scratch_s[:], scratch_s[:], AF.Exp)

              for oj in range(6, KHB):
                  nc.sync.dma_start(wk_s[:, oj * CW:(oj + 1) * CW], wk_e.ap()[oj])
              nc.sync.dma_start(ekn_s[:], ekn_e.ap())
              nc.sync.dma_start(xlo_s[:], xlo_e.ap())
              # v-phase stream: [wvh_oj, (amat), wvl_oj] per oj; amat chunks
              # slot in from oj=2 so fE_oj can chase; amat6/7 + wout go before
              # the final wvl so the tail is only v-pass3 work.
              AMAT_AT = {2: 0, 3: 1, 4: 2, 5: 3, 6: 4, 7: 5}
              for oj in range(KHB):
                  nc.sync.dma_start(wvh_s[:, oj * CW:(oj + 1) * CW], wvh_e.ap()[oj])
                  if oj in AMAT_AT:
                      aj = AMAT_AT[oj]
                      nc.sync.dma_start(a_s[:, aj * 6 * P:(aj + 1) * 6 * P],
                                        amat_e.ap()[aj])
                  if oj == KHB - 1:
                      nc.sync.dma_start(a_s[:, 6 * 6 * P:7 * 6 * P], amat_e.ap()[6])
                      nc.sync.dma_start(a_s[:, 7 * 6 * P:8 * 6 * P], amat_e.ap()[7])
                      nc.sync.dma_start(wout_s[:], wout_e.ap())
                  nc.sync.dma_start(wvl_s[:, oj * CW:(oj + 1) * CW], wvl_e.ap()[oj])

              xhi3 = xhi_s[:].rearrange("p (d two b) -> p d two b", d=DK, two=2)
              xlo3 = xlo_s[:].rearrange("p (d two b) -> p d two b", d=DK, two=2)

              def wblk(ws, oj, d):
                  base = oj * CW + d * 2 * P
                  return ws[:, base:base + 2 * P].rearrange(
                      "p (two m) -> p two m", two=2)

              # ---- phase 1: kT blocks (fp8 DR), evict +bias to fp8 kt,
              # square on DVE ----
              def k_block(oj):
                  ps = pkv.tile([P, BL], F32, tag="pkv")
                  for d in range(DK):
                      nc.tensor.matmul(ps[:], wblk(wk_s, oj, d), xhi3[:, d],
                                       start=(d == 0), stop=(d == DK - 1),
                                       perf_mode=DR)
                  sl = slice(oj * BL, (oj + 1) * BL)
                  nc.scalar.activation(kt_s[:, sl], ps[:], AF.Identity,
                                       bias=bkv_s[:, oj:oj + 1], scale=1.0 / WS)
                  nc.vector.tensor_mul(ksq_s[:, sl], kt_s[:, sl], kt_s[:, sl])

              for oj in range(KHB):
                  k_block(oj)

              # ---- phase 3 pieces (dots fp8 DR) ----
              kt3 = kt_s[:].rearrange("p (o b) -> p o b", o=KHB)
              ekn4 = ekn_s[:].rearrange("p (d two j) -> p d two j", d=D2, two=2)

              def dots(bc):
                  psd = pdot.tile([P, J], F32, tag="pdot")
                  for d in range(D2):
                      lhs = kt3[:, 2 * d:2 * d + 2, bc * P:(bc + 1) * P]
                      for (j0, nj) in JC:
                          nc.tensor.matmul(psd[:, j0:j0 + nj], lhs,
                                           ekn4[:, d, :, j0:j0 + nj],
                                           start=(d == 0), stop=(d == D2 - 1),
                                           perf_mode=DR)
                  return psd

              def softmax_chain(bc, psd):
                  e_sl = e_s[:, bc * J:(bc + 1) * J]
                  nc.scalar.activation(e_sl[:, 0:512], psd[:, 0:512], AF.Exp,
                                       scale=rinv_s[:, bc:bc + 1])
                  nc.scalar.activation(e_sl[:, 512:J], psd[:, 512:J], AF.Exp,
                                       scale=rinv_s[:, bc:bc + 1])
                  e3 = e_sl.rearrange("p (n m) -> p n m", m=M)
                  ncls_sl = slice(bc * NCLS, (bc + 1) * NCLS)
                  s2 = s_s[:, ncls_sl]
                  nc.vector.reduce_sum(s2, e3, axis=mybir.AxisListType.X)
                  nc.vector.tensor_mul(tmp_s[:], e_sl, evwb_s[:])
                  nc.vector.reduce_sum(num_s[:, ncls_sl],
                                       tmp_s[:].rearrange("p (n m) -> p n m", m=M),
                                       axis=mybir.AxisListType.X)
                  nc.vector.reciprocal(sinv_s[:, ncls_sl], s2)
                  nc.vector.tensor_mul(t_s[:, ncls_sl], num_s[:, ncls_sl],
                                       sinv_s[:, ncls_sl])
                  nc.scalar.activation(g_s[:, ncls_sl], t_s[:, ncls_sl], AF.Exp)
                  nc.vector.reduce_sum(gg_s[:, bc:bc + 1], g_s[:, ncls_sl],
                                       axis=mybir.AxisListType.X)
                  nc.vector.reciprocal(ginv_s[:, bc:bc + 1], gg_s[:, bc:bc + 1])
                  nc.vector.tensor_mul(c1_s[:, ncls_sl], g_s[:, ncls_sl],
                                       sinv_s[:, ncls_sl])
                  nc.vector.tensor_scalar_mul(c_s[:, ncls_sl], c1_s[:, ncls_sl],
                                              ginv_s[:, bc:bc + 1])
                  c_b = bass.AP(c_s.tensor, c_s[:, ncls_sl].offset,
                                c_s[:, ncls_sl].ap + [[0, M]])
                  u3 = u_s[:, bc * J:(bc + 1) * J].rearrange("p (n m) -> p n m", m=M)
                  nc.vector.tensor_mul(u3, e3, c_b)

              # ---- phase 2: sumsq via fp8 ones-matmul; rinv = rsqrt on DVE ----
              def sumsq():
                  ps2 = pmisc.tile([P, NB], F32, tag="misc")
                  for bc in range(NB):
                      for i in range(KHB):
                          nc.tensor.matmul(ps2[:, bc:bc + 1],
                                           ksq_s[:, i * BL + bc * P: i * BL + bc * P + P],
                                           ones_s[:],
                                           start=(i == 0), stop=(i == KHB - 1))
                  # rinv = rsqrt(sumsq) on DVE, vectorized over both batch
                  # chunks (magic + 1 Newton step, rel err ~2e-3 -- logits
                  # only reach ~0.03 so this is far below noise)
                  sq = rs1_s[:, 0:NB]
                  nc.vector.tensor_copy(sq, ps2[:, 0:NB])
                  y = rinv_s[:, 0:NB]
                  nc.vector.tensor_scalar(
                      y.bitcast(mybir.dt.int32), sq.bitcast(mybir.dt.int32),
                      1, None, op0=mybir.AluOpType.logical_shift_right)
                  nc.vector.tensor_tensor(
                      out=y.bitcast(mybir.dt.int32),
                      in0=bass.AP(magic_s.tensor, magic_s[:].offset,
                                  magic_s[:].ap[:1] + [[0, NB]]),
                      in1=y.bitcast(mybir.dt.int32),
                      op=mybir.AluOpType.subtract)
                  t1 = rs2_s[:, 0:NB]
                  nc.vector.tensor_mul(t1, y, y)
                  nc.vector.tensor_mul(t1, t1, sq)
                  nc.vector.tensor_scalar(t1, t1, -0.5, 1.5,
                                          op0=mybir.AluOpType.mult,
                                          op1=mybir.AluOpType.add)
                  nc.vector.tensor_mul(y, y, t1)

              # ---- transpose of u into ut (bf16, PE + DVE evictions) ----
              def transpose_u(bc):
                  def tgroup(grp):
                      pst = ptr.tile([P, 3 * P], BF, tag="ptr")
                      for t, jb in enumerate(grp):
                          w = JBS[jb]
                          nc.tensor.transpose(
                              pst[:w, t * P:(t + 1) * P],
                              u_s[:, bc * J + jb * P: bc * J + jb * P + w],
                              ident_s[:])
                      n = sum(1 for jb in grp if JBS[jb] == P)
                      base = ut_s[:, grp[0] * BL + bc * P: grp[0] * BL + bc * P + P]
                      dst = bass.AP(ut_s.tensor, base.offset,
                                    [base.ap[0], [BL, n], base.ap[1]])
                      nc.vector.tensor_copy(
                          dst, pst[:, 0:n * P].rearrange("p (n q) -> p n q", q=P))
                      if n < len(grp):
                          jb = grp[n]
                          w = JBS[jb]
                          nc.vector.tensor_copy(
                              ut_s[:w, jb * BL + bc * P: jb * BL + bc * P + P],
                              pst[:w, n * P:(n + 1) * P])
                  tgroup((0, 1, 2))
                  tgroup((3, 4, 5))

              # ---- v passes (fp8 DR) ----
              v_ps = {}

              def v12_block(oj):
                  ps = pkv.tile([P, BL], F32, tag="pkv")
                  v_ps[oj] = ps
                  for d in range(DK):
                      nc.tensor.matmul(ps[:], wblk(wvh_s, oj, d), xhi3[:, d],
                                       start=(d == 0), stop=False, perf_mode=DR)
                      nc.tensor.matmul(ps[:], wblk(wvh_s, oj, d), xlo3[:, d],
                                       start=False, stop=False, perf_mode=DR)

              def v3_block(oj):
                  ps = v_ps.pop(oj)
                  for d in range(DK):
                      nc.tensor.matmul(ps[:], wblk(wvl_s, oj, d), xhi3[:, d],
                                       start=False, stop=(d == DK - 1),
                                       perf_mode=DR)
                  nc.scalar.activation(hv_s[:, oj * BL:(oj + 1) * BL], ps[:],
                                       AF.Relu, bias=bkv_s[:, KHB + oj:KHB + oj + 1],
                                       scale=1.0 / WS)

              # ---- fE blocks (bf16) ----
              def fe_block(oj):
                  acc = pfe.tile([P, BL], F32, tag=f"pfe{oj % 2}")
                  for jb in range(6):
                      w = JBS[jb]
                      nc.tensor.matmul(
                          acc[:],
                          a_s[:w, oj * 6 * P + jb * P: oj * 6 * P + (jb + 1) * P],
                          ut_s[:w, jb * BL:(jb + 1) * BL],
                          start=(jb == 0), stop=(jb == 5))
                  dst = hfe_s[:, oj * BL:(oj + 1) * BL]
                  if oj % 2 == 0:
                      nc.scalar.activation(dst, acc[:], AF.Relu)
                  else:
                      nc.vector.tensor_scalar_max(dst, acc[:], 0.0)

              # ---- out accumulation (bf16), interleaved ----
              pso = pmisc.tile([NCLS, BL], F32, tag="misc")
              out_emitted = [0]

              def out_step(i, last=False):
                  h_s = hv_s if i < KHB else hfe_s
                  ii = i % KHB
                  nc.tensor.matmul(pso[:], wout_s[:, i * NCLS:(i + 1) * NCLS],
                                   h_s[:, ii * BL:(ii + 1) * BL],
                                   start=(out_emitted[0] == 0), stop=last)
                  out_emitted[0] += 1

              # ---- PE-ordered program ----
              # dots0 -> sumsq -> dots1 avoids a WAR deadlock: dots1 reuses
              # the pdot slot so it waits on exp0, which waits on rinv, which
              # needs the sumsq matmuls -- those must precede dots1 on PE.
              psd0 = dots(0)
              sumsq()
              psd1 = dots(1)
              softmax_chain(0, psd0)
              softmax_chain(1, psd1)
              v12_block(0)
              v3_block(0)
              v12_block(1)
              v3_block(1)
              transpose_u(0)
              transpose_u(1)
              for oj in range(2, KHB - 1):
                  v12_block(oj)
                  fe_block(oj - 2)
                  v3_block(oj)
                  # lag out steps: their h inputs were evicted earlier
                  out_step(oj - 2)                   # hv_{oj-2}
                  if oj >= 3:
                      out_step(KHB + oj - 3)         # hfe_{oj-3}
              # tail: fe5/6/7 run while wvl7 (the last transfer) is in
              # flight; only v3_7 + out steps remain after it lands.
              v12_block(KHB - 1)
              fe_block(5)
              out_step(5)                            # hv5
              out_step(KHB + 4)                      # hfe4
              fe_block(6)
              out_step(KHB + 5)                      # hfe5
              fe_block(7)
              out_step(KHB + 6)                      # hfe6
              v3_block(KHB - 1)
              out_step(6)                            # hv6
              out_step(KHB + 7)                      # hfe7
              out_step(7, last=True)                 # hv7

              nc.vector.tensor_scalar_add(out_sb[:], pso[:], bout_s[:, 0:1])
              nc.sync.dma_start(out_e.ap(), out_sb[:])

    nc.compile()
    return nc


def host_prep(x, static_feat, Wk, bk, Wv, bv, WEk, bEk, WEv, bEv, Ww, bw,
              Wout, bout):
    """Host-side fp32 precompute + per-core input maps."""
    EPS = 1e-8
    f32 = np.float32
    x = np.asarray(x, f32)
    static_feat = np.asarray(static_feat, f32)

    Ek = np.einsum('oc,ncm->nom', np.asarray(WEk, f32), static_feat,
                   optimize=True) + np.asarray(bEk, f32)[None, :, None]
    Ev = np.einsum('oc,ncm->nom', np.asarray(WEv, f32), static_feat,
                   optimize=True) + np.asarray(bEv, f32)[None, :, None]
    Ekn = Ek / np.maximum(np.linalg.norm(Ek, axis=1, keepdims=True), EPS)
    Ekn_mat = Ekn.transpose(1, 0, 2).reshape(CH, J)          # [CH, 672]
    A_mat = Ev.transpose(0, 2, 1).reshape(J, CH)             # [672, CH]
    evwb = np.einsum('nom,o->nm', Ev, np.asarray(Ww, f32)[0]).reshape(J)

    xT = np.ascontiguousarray(x[:, -1, :].T)                 # [CIN, B]
    xT_hi = xT.astype(f8)
    xT_lo = (xT - xT_hi.astype(f32)).astype(f8)

    def wsplit(W):
        # W [CH, CIN] -> WT*WS [CIN, CH] -> hi/lo fp8 (scaled out of
        # e4m3 subnormals; eviction divides by WS)
        WT = np.asarray(W, f32).T * WS
        hi = WT.astype(f8)
        lo = (WT - hi.astype(f32)).astype(f8)
        return hi, lo

    wk_hi, _ = wsplit(Wk)
    wv_hi, wv_lo = wsplit(Wv)

    def wlayout(WT8):
        # [CIN, CH] fp8 -> [KHB(oj), P, DK*2*P]: per (oj, d): [p, 2, 128m]
        a = WT8.reshape(DK, 2, P, KHB, P)        # (d, two, p, oj, m)
        return np.ascontiguousarray(
            a.transpose(3, 2, 0, 1, 4).reshape(KHB, P, DK * 2 * P))

    wk_h = wlayout(wk_hi)
    wvh_h = wlayout(wv_hi)
    wvl_h = wlayout(wv_lo)

    # ekn fp8: [CH, J] -> [P, D2*2*J]
    ekn8 = Ekn_mat.astype(f8)
    ekn_h = np.ascontiguousarray(
        ekn8.reshape(D2, 2, P, J).transpose(2, 0, 1, 3).reshape(P, D2 * 2 * J))

    # amat bf16: padded [768, CH] -> [KHB(oj), P, 6*P]
    a_pad = np.zeros((6 * P, CH), np.float32)
    a_pad[:J] = A_mat
    amat_h = np.ascontiguousarray(
        a_pad.reshape(6, P, KHB, P).transpose(2, 1, 0, 3).reshape(
            KHB, P, 6 * P)).astype(bf16)

    evwb_h = np.ascontiguousarray(
        np.broadcast_to(evwb.astype(bf16)[None, :], (P, J)))
    wout_h = np.ascontiguousarray(
        np.asarray(Wout, f32).T.reshape(KB, P, NCLS).transpose(1, 0, 2).reshape(
            P, KB * NCLS)).astype(bf16)
    bkv = np.concatenate([np.asarray(bk, f32), np.asarray(bv, f32)])
    bkv_h = np.ascontiguousarray(bkv.reshape(2 * KHB, P).T)
    bout_h = np.asarray(bout, f32).reshape(NCLS, 1)
    ident_h = np.eye(P, dtype=bf16)

    shared = dict(wk=wk_h, wvh=wvh_h, wvl=wvl_h, ekn=ekn_h, amat=amat_h,
                  evwb=evwb_h, wout=wout_h, bkv=bkv_h, bout=bout_h,
                  ident=ident_h)
    in_maps = []
    for c in range(NCORES):
        sl = slice(c * BL, (c + 1) * BL)

        def xlayout(x8):
            # [CIN, BL] fp8 -> [P, DK*2*BL]: per d: [p, 2, BL]
            a = x8[:, sl].reshape(DK, 2, P, BL)
            return np.ascontiguousarray(
                a.transpose(2, 0, 1, 3).reshape(P, DK * 2 * BL))

        in_maps.append(dict(xhi=xlayout(xT_hi), xlo=xlayout(xT_lo), **shared))
    return in_maps


_NC_CACHE = {}


def get_nc(debug=False, repeat=1):
    key = (debug, repeat)
    if key not in _NC_CACHE:
        _NC_CACHE[key] = build_nc(debug=debug, repeat=repeat)
    return _NC_CACHE[key]


def kernel(**inputs) -> np.ndarray:
    nc = get_nc()
    in_maps = host_prep(**inputs)
    res = run_bass_kernel_spmd(nc, in_maps, list(range(NCORES)))
    out = np.empty((B, NCLS, 1), dtype=np.float32)
    for c in range(NCORES):
        out[c * BL:(c + 1) * BL, :, 0] = res.results[c]["out"].T
    return out
